# revision 12
# baseline (speedup 1.0000x reference)
"""DCT heat-blur kernel for Trainium2 (8 NeuronCores, Bass/Tile).

Math: reference computes, per image X (one (batch, channel) slice):
    coefs = D X D^T;  coefs *= E;  out = D coefs D^T
with E[h,w] = exp(-(f_h^2 + f_w^2) t_b) = e e^T rank-1.  The elementwise
decay factors through the transforms:
    out = M X M^T,  M = D diag(e) D;  device computes W^T X W, W = M^T.
W_b is a tiny per-batch 256x256 matrix built on host.  The device does
2 GEMMs per image instead of 4 + an elementwise pass.

Device layout per 256x256 image: row-blocks a=0,1 of 128 rows each.
out = apply(apply(X, W), W) with apply(A, R) = A^T R via matmul.

Matmuls run in fp16 (full PE rate); I/O is fp16 BOTH directions -- the
host casts the fp16 result back to fp32.  Per-core DMA 21MB -> 14.7MB,
taking DMA off the critical path (PE throughput is the floor).

Startup latency mitigation: the NEFF has a ~7us engine-bootstrap
preamble before any dynamic DMA dispatch, and the PE clock-gate (HAM)
needs ~3.4us of sustained matmul activity to reach 2.4 GHz.  So (1) the
first images are loaded as SINGLE-image DMAs so image 0 lands ~1.5us
after dispatch instead of ~7us later, and (2) a bridge of small dummy
matmuls keeps the PE busy from the preamble until real data arrives, so
the array is already at full clock when the first real GEMM issues.

Sharding: pure data parallel over batch, 16 batches (48 images) per core.
"""

import os
import numpy as np

BATCH = 128
CHANNELS = 3
N = 256
N_CORES = 8
PB = BATCH // N_CORES          # batches per core
IMGS = PB * CHANNELS           # images per core
NSINGLE = 8                    # leading single-image loads
GRP = 4                        # images per DMA group after that
NWARM = 110                    # warmup bridge matmuls (N=128 each)

LAST_EXEC_TIME_NS = None
_NC_CACHE = {}


def _install_ntff_hook():
    """Wire antenv.axon_hooks (missing in this image) so trace=True works."""
    import sys
    import types

    if "antenv.axon_hooks" in sys.modules:
        return
    try:
        import trn_agent_boot.trn_boot as tb

        hook = tb._ntff_profile_via_ctypes("/opt/axon/libaxon_pjrt.so")
    except Exception:
        hook = None
    m = types.ModuleType("antenv.axon_hooks")
    m.get_axon_ntff_profile_hook = lambda: hook
    m.set_axon_ntff_profile_hook = lambda h: None
    sys.modules["antenv.axon_hooks"] = m


def _load_plan():
    """(start, count) spans: NSINGLE singles then GRP-sized groups."""
    plan = [(i, 1) for i in range(NSINGLE)]
    i = NSINGLE
    while i < IMGS:
        n = min(GRP, IMGS - i)
        plan.append((i, n))
        i += n
    return plan


def _build_nc():
    import concourse.bacc as bacc
    import concourse.tile as tile
    import concourse.mybir as mybir

    f32 = mybir.dt.float32
    f16 = mybir.dt.float16

    nc = bacc.Bacc("TRN2", target_bir_lowering=False, debug=False)
    # x/o host-permuted per image: [img][partition][rowblk, col]
    x_d = nc.dram_tensor("x", [IMGS, 128, 2 * N], f16, kind="ExternalInput").ap()
    w_d = nc.dram_tensor("w", [128, PB, 2, N], f16, kind="ExternalInput").ap()
    o_d = nc.dram_tensor("o", [IMGS, 128, 2 * N], f16, kind="ExternalOutput").ap()

    plan = _load_plan()

    with tile.TileContext(nc) as tc:
        with (
            tc.tile_pool(name="const", bufs=1) as cpool,
            tc.tile_pool(name="xpool", bufs=len(plan) + 1) as xpool,
            tc.tile_pool(name="tpool", bufs=6) as tpool,
            tc.tile_pool(name="opool", bufs=8) as opool,
            tc.tile_pool(name="ps1", bufs=3, space="PSUM") as ps1,
            tc.tile_pool(name="psw", bufs=1, space="PSUM") as psw,
            tc.tile_pool(name="ps2", bufs=4, space="PSUM") as ps2,
        ):
            # PE warmup bridge: small dummy matmuls fill the DMA-dead
            # preamble so the HAM clock-gate reaches 8/8 (2.4 GHz) and
            # STAYS there until real data arrives.  Fine granularity
            # (N=128) so the leftover queue drains fast once real
            # matmuls become ready.
            wu_sb = cpool.tile([128, 128], f16, name="wu")
            nc.gpsimd.memset(wu_sb[:], 0.25)
            wu_ps = psw.tile([128, 128], f32, name="wups")
            for _ in range(NWARM):
                nc.tensor.matmul(
                    wu_ps[:],
                    lhsT=wu_sb[:],
                    rhs=wu_sb[:],
                    start=True,
                    stop=True,
                )
            nc.vector.tensor_copy(out=wu_sb[:, 0:8], in_=wu_ps[:, 0:8])

            xt_tiles = {}

            def issue_load(gi):
                i0, cnt = plan[gi]
                xt = xpool.tile(
                    [128, cnt, 2, N], f16, tag="x", name=f"x{i0}",
                    padded_shape=[128, GRP, 2, N],
                )
                nc.sync.dma_start(
                    xt[:], x_d[i0 : i0 + cnt].rearrange("i p (a w) -> p i a w", a=2)
                )
                xt_tiles[gi] = xt

            # W quarters; first quarter + first image on the early queue
            w_q = []
            with tc.high_priority():
                wq0 = cpool.tile([128, 4, 2, N], f16, name="wq0")
                nc.sync.dma_start(wq0[:], w_d[:, 0:4])
                w_q.append(wq0)
                issue_load(0)
            for q in range(1, 4):
                wq = cpool.tile([128, 4, 2, N], f16, name=f"wq{q}")
                nc.scalar.dma_start(wq[:], w_d[:, 4 * q : 4 * (q + 1)])
                w_q.append(wq)
            for gi in range(1, len(plan)):
                issue_load(gi)

            img = 0
            for gi, (i0, cnt) in enumerate(plan):
                xt = xt_tiles.pop(gi)
                for ii in range(cnt):
                    img = i0 + ii
                    b = img // CHANNELS
                    wv = w_q[b // 4][:, b % 4]
                    t1_ps = ps1.tile([128, 2, N], f32)
                    for mb in range(2):
                        for a in range(2):
                            nc.tensor.matmul(
                                t1_ps[:, mb, :],
                                lhsT=xt[:, ii, a, mb * 128 : (mb + 1) * 128],
                                rhs=wv[:, a],
                                start=(a == 0),
                                stop=(a == 1),
                            )
                    t1_sb = tpool.tile([128, 2, N], f16)
                    if img % 2 == 0:
                        nc.vector.tensor_copy(out=t1_sb[:], in_=t1_ps[:])
                    else:
                        nc.scalar.copy(t1_sb[:], t1_ps[:])
                    t2_ps = ps2.tile([128, 2, N], f32)
                    for mb in range(2):
                        for a in range(2):
                            nc.tensor.matmul(
                                t2_ps[:, mb, :],
                                lhsT=t1_sb[:, a, mb * 128 : (mb + 1) * 128],
                                rhs=wv[:, a],
                                start=(a == 0),
                                stop=(a == 1),
                            )
                    ot = opool.tile([128, 2, N], f16, tag="o", name=f"o{img}")
                    if img % 2 == 0:
                        nc.scalar.copy(ot[:], t2_ps[:])
                    else:
                        nc.vector.tensor_copy(out=ot[:], in_=t2_ps[:])
                    st = nc.sync if img % 2 == 0 else nc.gpsimd
                    st.dma_start(
                        o_d[img].rearrange("p (a w) -> p a w", a=2), ot[:]
                    )

    nc.compile()
    return nc


def _get_nc():
    key = "nc_v4"
    if key not in _NC_CACHE:
        _NC_CACHE[key] = _build_nc()
    return _NC_CACHE[key]


def _host_w(blur_sigmas, fwd_steps):
    """Per-batch W_b = (D diag(e_b) D)^T in device layout [128, B, 2, N]."""
    sig = np.asarray(blur_sigmas, dtype=np.float64)
    steps = np.asarray(fwd_steps).astype(np.int64)
    n = np.arange(N, dtype=np.float64)
    D = np.sqrt(2.0 / N) * np.cos(np.pi * (n[None, :] + 0.5) * n[:, None] / N)
    D[0] *= 1.0 / np.sqrt(2.0)
    freqs = np.pi * n / N
    uniq, inv = np.unique(steps, return_inverse=True)
    ms = np.empty((len(uniq), N, N), dtype=np.float16)
    for i, s in enumerate(uniq):
        t = sig[s] ** 2 / 2.0
        e = np.exp(-(freqs**2) * t)
        w = (D @ (e[:, None] * D)).T
        ms[i] = w.astype(np.float16)
    w_all = ms[inv]  # [B, N, N]
    # device layout [128, B, 2, N]: [p, b, a, h] = W_b[a*128+p, h]
    return np.ascontiguousarray(
        w_all.reshape(BATCH, 2, 128, N).transpose(2, 0, 1, 3)
    )


def kernel(x, blur_sigmas, fwd_steps):
    global LAST_EXEC_TIME_NS
    from concourse import bass_utils

    x = np.asarray(x)
    assert x.shape == (BATCH, CHANNELS, N, N), x.shape
    x = x.astype(np.float16)
    w_host = _host_w(blur_sigmas, fwd_steps)

    # device x layout per core: [IMGS, 128, 2*N]; x[img, a*128+p, w]
    xp = (
        x.reshape(N_CORES, IMGS, 2, 128, N)
        .transpose(0, 1, 3, 2, 4)
        .reshape(N_CORES, IMGS, 128, 2 * N)
    )
    in_maps = []
    for i in range(N_CORES):
        in_maps.append(
            {
                "x": np.ascontiguousarray(xp[i]),
                "w": np.ascontiguousarray(w_host[:, i * PB : (i + 1) * PB]),
            }
        )

    nc = _get_nc()
    trace = os.environ.get("BASS_DCT_TRACE", "0") == "1"
    kwargs = {}
    if trace:
        _install_ntff_hook()
        kwargs["trace"] = True
        tmpdir = os.environ.get("BASS_DCT_TRACE_DIR")
        if tmpdir:
            kwargs["tmpdir"] = tmpdir
    res = None
    for attempt in range(3):
        try:
            res = bass_utils.run_bass_kernel_spmd(
                nc, in_maps, core_ids=list(range(N_CORES)), **kwargs
            )
            break
        except Exception:
            # transient NRT_EXEC_UNIT_UNRECOVERABLE has been observed on the
            # first execution of a freshly loaded NEFF; a retry succeeds
            if attempt == 2:
                raise
            import time as _time

            _time.sleep(2.0)
            kwargs.pop("trace", None)
            kwargs.pop("tmpdir", None)
    LAST_EXEC_TIME_NS = res.exec_time_ns

    # inverse permute: oc[img, p, (a, w)] -> out[img, a*128+p, w]
    oc = np.stack([res.results[i]["o"] for i in range(N_CORES)])
    out = (
        oc.reshape(N_CORES, IMGS, 128, 2, N)
        .transpose(0, 1, 3, 2, 4)
        .reshape(BATCH, CHANNELS, N, N)
    )
    return np.ascontiguousarray(out.astype(np.float32))


# revision 13
# speedup vs baseline: 1.2990x; 1.2990x over previous
"""DCT heat-blur kernel for Trainium2 (8 NeuronCores, Bass/Tile).

Math: reference computes, per image X (one (batch, channel) slice):
    coefs = D X D^T;  coefs *= E;  out = D coefs D^T
with E[h,w] = exp(-(f_h^2 + f_w^2) t_b) = e e^T rank-1.  The elementwise
decay factors through the transforms:
    out = M X M^T,  M = D diag(e) D;  device computes W^T X W, W = M^T.
W_b is a tiny per-batch 256x256 matrix built on host.  The device does
2 GEMMs per image instead of 4 + an elementwise pass.

Device layout per 256x256 image: row-blocks a=0,1 of 128 rows each.
out = apply(apply(X, W), W) with apply(A, R) = A^T R via matmul.

Matmuls run in fp16 (full PE rate); I/O is fp16 BOTH directions -- the
host casts the fp16 result back to fp32.  Per-core DMA 21MB -> 14.7MB,
taking DMA off the critical path (PE throughput is the floor).

Startup mitigation (the NEFF spends ~7us in engine bootstrap before any
dynamic DMA, and the PE clock-gate needs ~3.4us of sustained activity to
reach 2.4 GHz):
 1. ALL loads go on one ring in need-order (W quarter 0, x groups 0-1,
    then later W quarters interleaved between x groups) so the first
    image + its W complete ~2.5us after DMA spin-up instead of fair-
    sharing bandwidth with 2MB of W needed much later.
 2. A bridge of small dummy matmuls keeps the PE busy from bootstrap
    until real data arrives, so the HAM clock-gate is already 8/8 when
    the first real GEMM issues and never re-throttles.

Sharding: pure data parallel over batch, 16 batches (48 images) per core.
"""

import os
import numpy as np

BATCH = 128
CHANNELS = 3
N = 256
N_CORES = 8
PB = BATCH // N_CORES          # batches per core
IMGS = PB * CHANNELS           # images per core
GRP = 4                        # images per DMA group
NG = IMGS // GRP               # groups per core
NWARM = 45                     # warmup bridge matmuls (N=128 each)

LAST_EXEC_TIME_NS = None
_NC_CACHE = {}


def _install_ntff_hook():
    """Wire antenv.axon_hooks (missing in this image) so trace=True works."""
    import sys
    import types

    if "antenv.axon_hooks" in sys.modules:
        return
    try:
        import trn_agent_boot.trn_boot as tb

        hook = tb._ntff_profile_via_ctypes("/opt/axon/libaxon_pjrt.so")
    except Exception:
        hook = None
    m = types.ModuleType("antenv.axon_hooks")
    m.get_axon_ntff_profile_hook = lambda: hook
    m.set_axon_ntff_profile_hook = lambda h: None
    sys.modules["antenv.axon_hooks"] = m


def _build_nc():
    import concourse.bacc as bacc
    import concourse.tile as tile
    import concourse.mybir as mybir

    f32 = mybir.dt.float32
    f16 = mybir.dt.float16

    nc = bacc.Bacc("TRN2", target_bir_lowering=False, debug=False)
    # x/o are host-permuted: [group][partition][img_in_grp, rowblk, col]
    x_d = nc.dram_tensor("x", [NG, 128, GRP * 2 * N], f16, kind="ExternalInput").ap()
    # w: host-built per-batch W matrices, [partition][batch, rowblk, col]
    w_d = nc.dram_tensor("w", [128, PB, 2, N], f16, kind="ExternalInput").ap()
    o_d = nc.dram_tensor("o", [NG, 128, GRP * 2 * N], f16, kind="ExternalOutput").ap()

    with tile.TileContext(nc) as tc:
        with (
            tc.tile_pool(name="const", bufs=1) as cpool,
            tc.tile_pool(name="xpool", bufs=NG + 1) as xpool,
            tc.tile_pool(name="tpool", bufs=6) as tpool,
            tc.tile_pool(name="opool", bufs=8) as opool,
            tc.tile_pool(name="ps1", bufs=3, space="PSUM") as ps1,
            tc.tile_pool(name="psw", bufs=1, space="PSUM") as psw,
            tc.tile_pool(name="ps2", bufs=4, space="PSUM") as ps2,
        ):
            # PE warmup bridge (see module docstring)
            wu_sb = cpool.tile([128, 128], f16, name="wu")
            nc.gpsimd.memset(wu_sb[:], 0.25)
            wu_ps = psw.tile([128, 128], f32, name="wups")
            for _ in range(NWARM):
                nc.tensor.matmul(
                    wu_ps[:], lhsT=wu_sb[:], rhs=wu_sb[:], start=True, stop=True
                )
            nc.vector.tensor_copy(out=wu_sb[:, 0:8], in_=wu_ps[:, 0:8])

            xt_tiles = {}
            w_q = {}

            def issue_load(g):
                xt = xpool.tile([128, GRP, 2, N], f16)
                nc.sync.dma_start(
                    xt[:], x_d[g].rearrange("p (i a w) -> p i a w", i=GRP, a=2)
                )
                xt_tiles[g] = xt

            def issue_wq(q):
                wq = cpool.tile([128, 4, 2, N], f16, name=f"wq{q}")
                nc.sync.dma_start(wq[:], w_d[:, 4 * q : 4 * (q + 1)])
                w_q[q] = wq

            # one ring, need-order: everything arrives roughly in the
            # order compute consumes it
            issue_wq(0)
            issue_load(0)
            issue_load(1)
            issue_wq(1)
            issue_load(2)
            issue_load(3)
            issue_wq(2)
            issue_load(4)
            issue_load(5)
            issue_wq(3)
            for g in range(6, NG):
                issue_load(g)

            for g in range(NG):
                xt = xt_tiles.pop(g)
                ot = opool.tile([128, GRP, 2, N], f16)
                for ii in range(GRP):
                    img = g * GRP + ii
                    b = img // CHANNELS
                    wv = w_q[b // 4][:, b % 4]
                    t1_ps = ps1.tile([128, 2, N], f32)
                    for mb in range(2):
                        for a in range(2):
                            nc.tensor.matmul(
                                t1_ps[:, mb, :],
                                lhsT=xt[:, ii, a, mb * 128 : (mb + 1) * 128],
                                rhs=wv[:, a],
                                start=(a == 0),
                                stop=(a == 1),
                            )
                    t1_sb = tpool.tile([128, 2, N], f16)
                    if ii % 2 == 0:
                        nc.vector.tensor_copy(out=t1_sb[:], in_=t1_ps[:])
                    else:
                        nc.scalar.copy(t1_sb[:], t1_ps[:])
                    t2_ps = ps2.tile([128, 2, N], f32)
                    for mb in range(2):
                        for a in range(2):
                            nc.tensor.matmul(
                                t2_ps[:, mb, :],
                                lhsT=t1_sb[:, a, mb * 128 : (mb + 1) * 128],
                                rhs=wv[:, a],
                                start=(a == 0),
                                stop=(a == 1),
                            )
                    if ii % 2 == 0:
                        nc.scalar.copy(ot[:, ii], t2_ps[:])
                    else:
                        nc.vector.tensor_copy(out=ot[:, ii], in_=t2_ps[:])
                    # late groups store per image so the final drain is
                    # small pieces that overlap the last compute
                    if g >= NG // 2:
                        nc.gpsimd.dma_start(
                            o_d[g].rearrange(
                                "p (i a w) -> p i a w", i=GRP, a=2
                            )[:, ii],
                            ot[:, ii],
                        )
                if g < NG // 2:
                    nc.scalar.dma_start(
                        o_d[g].rearrange("p (i a w) -> p i a w", i=GRP, a=2), ot[:]
                    )

    nc.compile()
    return nc


def _get_nc():
    key = "nc_v5"
    if key not in _NC_CACHE:
        _NC_CACHE[key] = _build_nc()
    return _NC_CACHE[key]


def _host_w(blur_sigmas, fwd_steps):
    """Per-batch W_b = (D diag(e_b) D)^T in device layout [128, B, 2, N]."""
    sig = np.asarray(blur_sigmas, dtype=np.float64)
    steps = np.asarray(fwd_steps).astype(np.int64)
    n = np.arange(N, dtype=np.float64)
    D = np.sqrt(2.0 / N) * np.cos(np.pi * (n[None, :] + 0.5) * n[:, None] / N)
    D[0] *= 1.0 / np.sqrt(2.0)
    freqs = np.pi * n / N
    uniq, inv = np.unique(steps, return_inverse=True)
    ms = np.empty((len(uniq), N, N), dtype=np.float16)
    for i, s in enumerate(uniq):
        t = sig[s] ** 2 / 2.0
        e = np.exp(-(freqs**2) * t)
        w = (D @ (e[:, None] * D)).T
        ms[i] = w.astype(np.float16)
    w_all = ms[inv]  # [B, N, N]
    # device layout [128, B, 2, N]: [p, b, a, h] = W_b[a*128+p, h]
    return np.ascontiguousarray(
        w_all.reshape(BATCH, 2, 128, N).transpose(2, 0, 1, 3)
    )


def kernel(x, blur_sigmas, fwd_steps):
    global LAST_EXEC_TIME_NS
    from concourse import bass_utils

    x = np.asarray(x)
    assert x.shape == (BATCH, CHANNELS, N, N), x.shape
    x = x.astype(np.float16)
    w_host = _host_w(blur_sigmas, fwd_steps)

    # device x layout: [core][NG, 128, GRP*2*N]
    # x[img, a*128+p, w] -> xc[g, p, (i, a, w)]
    xp = (
        x.reshape(N_CORES, NG, GRP, 2, 128, N)
        .transpose(0, 1, 4, 2, 3, 5)
        .reshape(N_CORES, NG, 128, GRP * 2 * N)
    )
    in_maps = []
    for i in range(N_CORES):
        in_maps.append(
            {
                "x": np.ascontiguousarray(xp[i]),
                "w": np.ascontiguousarray(w_host[:, i * PB : (i + 1) * PB]),
            }
        )

    nc = _get_nc()
    trace = os.environ.get("BASS_DCT_TRACE", "0") == "1"
    kwargs = {}
    if trace:
        _install_ntff_hook()
        kwargs["trace"] = True
        tmpdir = os.environ.get("BASS_DCT_TRACE_DIR")
        if tmpdir:
            kwargs["tmpdir"] = tmpdir
    res = None
    for attempt in range(3):
        try:
            res = bass_utils.run_bass_kernel_spmd(
                nc, in_maps, core_ids=list(range(N_CORES)), **kwargs
            )
            break
        except Exception:
            # transient NRT_EXEC_UNIT_UNRECOVERABLE has been observed on the
            # first execution of a freshly loaded NEFF; a retry succeeds
            if attempt == 2:
                raise
            import time as _time

            _time.sleep(2.0)
            kwargs.pop("trace", None)
            kwargs.pop("tmpdir", None)
    LAST_EXEC_TIME_NS = res.exec_time_ns

    # inverse permute: oc[g, p, (i, a, w)] -> out[img, a*128+p, w]
    oc = np.stack([res.results[i]["o"] for i in range(N_CORES)])
    out = (
        oc.reshape(N_CORES, NG, 128, GRP, 2, N)
        .transpose(0, 1, 3, 4, 2, 5)
        .reshape(BATCH, CHANNELS, N, N)
    )
    return np.ascontiguousarray(out.astype(np.float32))


# revision 14
# speedup vs baseline: 1.3213x; 1.0172x over previous
"""DCT heat-blur kernel for Trainium2 (8 NeuronCores, Bass/Tile).

Math: reference computes, per image X (one (batch, channel) slice):
    coefs = D X D^T;  coefs *= E;  out = D coefs D^T
with E[h,w] = exp(-(f_h^2 + f_w^2) t_b) = e e^T rank-1.  The elementwise
decay factors through the transforms:
    out = M X M^T,  M = D diag(e) D;  device computes W^T X W, W = M^T.
W_b is a tiny per-batch 256x256 matrix built on host.  The device does
2 GEMMs per image instead of 4 + an elementwise pass.

Device layout per 256x256 image: row-blocks a=0,1 of 128 rows each.
out = apply(apply(X, W), W) with apply(A, R) = A^T R via matmul.

Matmuls run in fp16 (full PE rate); I/O is fp16 BOTH directions -- the
host casts the fp16 result back to fp32.  Per-core DMA 21MB -> 14.7MB,
taking DMA off the critical path (PE throughput is the floor).

Startup mitigation (the NEFF spends ~7us in engine bootstrap before any
dynamic DMA, and the PE clock-gate needs ~3.4us of sustained activity to
reach 2.4 GHz):
 1. ALL loads go on one ring in need-order (W quarter 0, x groups 0-1,
    then later W quarters interleaved between x groups) so the first
    image + its W complete ~2.5us after DMA spin-up instead of fair-
    sharing bandwidth with 2MB of W needed much later.
 2. A bridge of small dummy matmuls keeps the PE busy from bootstrap
    until real data arrives, so the HAM clock-gate is already 8/8 when
    the first real GEMM issues and never re-throttles.

Sharding: pure data parallel over batch, 16 batches (48 images) per core.
"""

import os
import numpy as np

BATCH = 128
CHANNELS = 3
N = 256
N_CORES = 8
PB = BATCH // N_CORES          # batches per core
IMGS = PB * CHANNELS           # images per core
GRP = 4                        # images per DMA group
NG = IMGS // GRP               # groups per core
NWARM = 40                     # warmup bridge matmuls (N=128 each)

LAST_EXEC_TIME_NS = None
_NC_CACHE = {}


def _install_ntff_hook():
    """Wire antenv.axon_hooks (missing in this image) so trace=True works."""
    import sys
    import types

    if "antenv.axon_hooks" in sys.modules:
        return
    try:
        import trn_agent_boot.trn_boot as tb

        hook = tb._ntff_profile_via_ctypes("/opt/axon/libaxon_pjrt.so")
    except Exception:
        hook = None
    m = types.ModuleType("antenv.axon_hooks")
    m.get_axon_ntff_profile_hook = lambda: hook
    m.set_axon_ntff_profile_hook = lambda h: None
    sys.modules["antenv.axon_hooks"] = m


def _build_nc():
    import concourse.bacc as bacc
    import concourse.tile as tile
    import concourse.mybir as mybir

    f32 = mybir.dt.float32
    f16 = mybir.dt.float16

    nc = bacc.Bacc("TRN2", target_bir_lowering=False, debug=False)
    # x/o are host-permuted: [group][partition][img_in_grp, rowblk, col]
    x_d = nc.dram_tensor("x", [NG, 128, GRP * 2 * N], f16, kind="ExternalInput").ap()
    # w: host-built per-batch W matrices, [partition][batch, rowblk, col]
    w_d = nc.dram_tensor("w", [128, PB, 2, N], f16, kind="ExternalInput").ap()
    o_d = nc.dram_tensor("o", [NG, 128, GRP * 2 * N], f16, kind="ExternalOutput").ap()

    with tile.TileContext(nc) as tc:
        with (
            tc.tile_pool(name="const", bufs=1) as cpool,
            tc.tile_pool(name="xpool", bufs=NG + 1) as xpool,
            tc.tile_pool(name="tpool", bufs=6) as tpool,
            tc.tile_pool(name="opool", bufs=8) as opool,
            tc.tile_pool(name="ps1", bufs=3, space="PSUM") as ps1,
            tc.tile_pool(name="psw", bufs=1, space="PSUM") as psw,
            tc.tile_pool(name="ps2", bufs=4, space="PSUM") as ps2,
        ):
            # PE warmup bridge (see module docstring)
            wu_sb = cpool.tile([128, 128], f16, name="wu")
            nc.gpsimd.memset(wu_sb[:], 0.25)
            wu_ps = psw.tile([128, 128], f32, name="wups")
            for _ in range(NWARM):
                nc.tensor.matmul(
                    wu_ps[:], lhsT=wu_sb[:], rhs=wu_sb[:], start=True, stop=True
                )
            nc.vector.tensor_copy(out=wu_sb[:, 0:8], in_=wu_ps[:, 0:8])

            xt_tiles = {}
            w_q = {}

            def issue_load(g):
                xt = xpool.tile([128, GRP, 2, N], f16)
                nc.sync.dma_start(
                    xt[:], x_d[g].rearrange("p (i a w) -> p i a w", i=GRP, a=2)
                )
                xt_tiles[g] = xt

            def issue_wq(q):
                wq = cpool.tile([128, 2, 2, N], f16, name=f"wq{q}")
                nc.sync.dma_start(wq[:], w_d[:, 2 * q : 2 * (q + 1)])
                w_q[q] = wq

            xh_tiles = {}

            def issue_load_half(g, h):
                xt = xpool.tile(
                    [128, 2, 2, N], f16, tag="xh", name=f"xh{g}_{h}",
                    padded_shape=[128, GRP, 2, N],
                )
                nc.sync.dma_start(
                    xt[:],
                    x_d[g].rearrange("p (i a w) -> p i a w", i=GRP, a=2)[
                        :, 2 * h : 2 * h + 2
                    ],
                )
                xh_tiles[(g, h)] = xt

            # one ring, need-order: everything arrives roughly in the
            # order compute consumes it; the first pieces are halved so
            # the very first image + its W complete as early as possible
            issue_wq(0)
            issue_load_half(0, 0)
            issue_load_half(0, 1)
            issue_wq(1)
            issue_load_half(1, 0)
            issue_load_half(1, 1)
            issue_wq(2)
            issue_load(2)
            issue_wq(3)
            issue_load(3)
            issue_wq(4)
            issue_load(4)
            issue_wq(5)
            issue_load(5)
            issue_wq(6)
            issue_load(6)
            issue_wq(7)
            for g in range(7, NG):
                issue_load(g)

            for g in range(NG):
                ot = opool.tile([128, GRP, 2, N], f16)
                for ii in range(GRP):
                    img = g * GRP + ii
                    b = img // CHANNELS
                    wv = w_q[b // 2][:, b % 2]
                    if g < 2:
                        xt = xh_tiles[(g, ii // 2)][:, ii % 2 : ii % 2 + 1]
                    else:
                        xt = xt_tiles[g][:, ii : ii + 1]
                    t1_ps = ps1.tile([128, 2, N], f32)
                    for mb in range(2):
                        for a in range(2):
                            nc.tensor.matmul(
                                t1_ps[:, mb, :],
                                lhsT=xt[:, 0, a, mb * 128 : (mb + 1) * 128],
                                rhs=wv[:, a],
                                start=(a == 0),
                                stop=(a == 1),
                            )
                    t1_sb = tpool.tile([128, 2, N], f16)
                    if ii % 2 == 0:
                        nc.vector.tensor_copy(out=t1_sb[:], in_=t1_ps[:])
                    else:
                        nc.scalar.copy(t1_sb[:], t1_ps[:])
                    t2_ps = ps2.tile([128, 2, N], f32)
                    for mb in range(2):
                        for a in range(2):
                            nc.tensor.matmul(
                                t2_ps[:, mb, :],
                                lhsT=t1_sb[:, a, mb * 128 : (mb + 1) * 128],
                                rhs=wv[:, a],
                                start=(a == 0),
                                stop=(a == 1),
                            )
                    if ii % 2 == 0:
                        nc.scalar.copy(ot[:, ii], t2_ps[:])
                    else:
                        nc.vector.tensor_copy(out=ot[:, ii], in_=t2_ps[:])
                    # late groups store per image-pair on the two idle
                    # rings so the final drain overlaps the last compute
                    if g >= NG // 2 and ii % 2 == 1:
                        st = nc.gpsimd if (img // 2) % 2 == 0 else nc.sync
                        st.dma_start(
                            o_d[g].rearrange(
                                "p (i a w) -> p i a w", i=GRP, a=2
                            )[:, ii - 1 : ii + 1],
                            ot[:, ii - 1 : ii + 1],
                        )
                if g < NG // 2:
                    nc.scalar.dma_start(
                        o_d[g].rearrange("p (i a w) -> p i a w", i=GRP, a=2), ot[:]
                    )

    nc.compile()
    return nc


def _get_nc():
    key = "nc_v6"
    if key not in _NC_CACHE:
        _NC_CACHE[key] = _build_nc()
    return _NC_CACHE[key]


def _host_w(blur_sigmas, fwd_steps):
    """Per-batch W_b = (D diag(e_b) D)^T in device layout [128, B, 2, N]."""
    sig = np.asarray(blur_sigmas, dtype=np.float64)
    steps = np.asarray(fwd_steps).astype(np.int64)
    n = np.arange(N, dtype=np.float64)
    D = np.sqrt(2.0 / N) * np.cos(np.pi * (n[None, :] + 0.5) * n[:, None] / N)
    D[0] *= 1.0 / np.sqrt(2.0)
    freqs = np.pi * n / N
    uniq, inv = np.unique(steps, return_inverse=True)
    ms = np.empty((len(uniq), N, N), dtype=np.float16)
    for i, s in enumerate(uniq):
        t = sig[s] ** 2 / 2.0
        e = np.exp(-(freqs**2) * t)
        w = (D @ (e[:, None] * D)).T
        ms[i] = w.astype(np.float16)
    w_all = ms[inv]  # [B, N, N]
    # device layout [128, B, 2, N]: [p, b, a, h] = W_b[a*128+p, h]
    return np.ascontiguousarray(
        w_all.reshape(BATCH, 2, 128, N).transpose(2, 0, 1, 3)
    )


def kernel(x, blur_sigmas, fwd_steps):
    global LAST_EXEC_TIME_NS
    from concourse import bass_utils

    x = np.asarray(x)
    assert x.shape == (BATCH, CHANNELS, N, N), x.shape
    x = x.astype(np.float16)
    w_host = _host_w(blur_sigmas, fwd_steps)

    # device x layout: [core][NG, 128, GRP*2*N]
    # x[img, a*128+p, w] -> xc[g, p, (i, a, w)]
    xp = (
        x.reshape(N_CORES, NG, GRP, 2, 128, N)
        .transpose(0, 1, 4, 2, 3, 5)
        .reshape(N_CORES, NG, 128, GRP * 2 * N)
    )
    in_maps = []
    for i in range(N_CORES):
        in_maps.append(
            {
                "x": np.ascontiguousarray(xp[i]),
                "w": np.ascontiguousarray(w_host[:, i * PB : (i + 1) * PB]),
            }
        )

    nc = _get_nc()
    trace = os.environ.get("BASS_DCT_TRACE", "0") == "1"
    kwargs = {}
    if trace:
        _install_ntff_hook()
        kwargs["trace"] = True
        tmpdir = os.environ.get("BASS_DCT_TRACE_DIR")
        if tmpdir:
            kwargs["tmpdir"] = tmpdir
    res = None
    for attempt in range(3):
        try:
            res = bass_utils.run_bass_kernel_spmd(
                nc, in_maps, core_ids=list(range(N_CORES)), **kwargs
            )
            break
        except Exception:
            # transient NRT_EXEC_UNIT_UNRECOVERABLE has been observed on the
            # first execution of a freshly loaded NEFF; a retry succeeds
            if attempt == 2:
                raise
            import time as _time

            _time.sleep(2.0)
            kwargs.pop("trace", None)
            kwargs.pop("tmpdir", None)
    LAST_EXEC_TIME_NS = res.exec_time_ns

    # inverse permute: oc[g, p, (i, a, w)] -> out[img, a*128+p, w]
    oc = np.stack([res.results[i]["o"] for i in range(N_CORES)])
    out = (
        oc.reshape(N_CORES, NG, 128, GRP, 2, N)
        .transpose(0, 1, 3, 4, 2, 5)
        .reshape(BATCH, CHANNELS, N, N)
    )
    return np.ascontiguousarray(out.astype(np.float32))


# revision 15
# speedup vs baseline: 1.3475x; 1.0198x over previous
"""DCT heat-blur kernel for Trainium2 (8 NeuronCores, Bass/Tile).

Math: reference computes, per image X (one (batch, channel) slice):
    coefs = D X D^T;  coefs *= E;  out = D coefs D^T
with E[h,w] = exp(-(f_h^2 + f_w^2) t_b) = e e^T rank-1.  The elementwise
decay factors through the transforms:
    out = M X M^T,  M = D diag(e) D;  device computes W^T X W, W = M^T.
W_b is a tiny per-batch 256x256 matrix built on host.  The device does
2 GEMMs per image instead of 4 + an elementwise pass.

Device layout per 256x256 image: row-blocks a=0,1 of 128 rows each.
out = apply(apply(X, W), W) with apply(A, R) = A^T R via matmul.

Matmuls run in fp16 (full PE rate); I/O is fp16 BOTH directions -- the
host casts the fp16 result back to fp32.  Per-core DMA 21MB -> 14.7MB,
taking DMA off the critical path (PE throughput is the floor).

Startup mitigation (the NEFF spends ~7us in engine bootstrap before any
dynamic DMA, and the PE clock-gate needs ~3.4us of sustained activity to
reach 2.4 GHz):
 1. ALL loads go on one ring in need-order (W quarter 0, x groups 0-1,
    then later W quarters interleaved between x groups) so the first
    image + its W complete ~2.5us after DMA spin-up instead of fair-
    sharing bandwidth with 2MB of W needed much later.
 2. A bridge of small dummy matmuls keeps the PE busy from bootstrap
    until real data arrives, so the HAM clock-gate is already 8/8 when
    the first real GEMM issues and never re-throttles.

Sharding: pure data parallel over batch, 16 batches (48 images) per core.
"""

import os
import numpy as np

BATCH = 128
CHANNELS = 3
N = 256
N_CORES = 8
PB = BATCH // N_CORES          # batches per core
IMGS = PB * CHANNELS           # images per core
GRP = 4                        # images per DMA group
NG = IMGS // GRP               # groups per core
NWARM = 40                     # warmup bridge matmuls (N=128 each)

LAST_EXEC_TIME_NS = None
_NC_CACHE = {}


def _install_ntff_hook():
    """Wire antenv.axon_hooks (missing in this image) so trace=True works."""
    import sys
    import types

    if "antenv.axon_hooks" in sys.modules:
        return
    try:
        import trn_agent_boot.trn_boot as tb

        hook = tb._ntff_profile_via_ctypes("/opt/axon/libaxon_pjrt.so")
    except Exception:
        hook = None
    m = types.ModuleType("antenv.axon_hooks")
    m.get_axon_ntff_profile_hook = lambda: hook
    m.set_axon_ntff_profile_hook = lambda h: None
    sys.modules["antenv.axon_hooks"] = m


def _build_nc():
    import concourse.bacc as bacc
    import concourse.tile as tile
    import concourse.mybir as mybir

    f32 = mybir.dt.float32
    f16 = mybir.dt.float16

    nc = bacc.Bacc("TRN2", target_bir_lowering=False, debug=False)
    # x/o are host-permuted: [group][partition][img_in_grp, rowblk, col]
    x_d = nc.dram_tensor("x", [NG, 128, GRP * 2 * N], f16, kind="ExternalInput").ap()
    # w: host-built per-batch W matrices, [partition][batch, rowblk, col]
    w_d = nc.dram_tensor("w", [128, PB, 2, N], f16, kind="ExternalInput").ap()
    o_d = nc.dram_tensor("o", [NG, 128, GRP * 2 * N], f16, kind="ExternalOutput").ap()

    with tile.TileContext(nc) as tc:
        with (
            tc.tile_pool(name="const", bufs=1) as cpool,
            tc.tile_pool(name="xpool", bufs=NG + 1) as xpool,
            tc.tile_pool(name="tpool", bufs=6) as tpool,
            tc.tile_pool(name="opool", bufs=8) as opool,
            tc.tile_pool(name="ps1", bufs=3, space="PSUM") as ps1,
            tc.tile_pool(name="psw", bufs=1, space="PSUM") as psw,
            tc.tile_pool(name="ps2", bufs=4, space="PSUM") as ps2,
        ):
            # PE warmup bridge (see module docstring)
            wu_sb = cpool.tile([128, 128], f16, name="wu")
            nc.gpsimd.memset(wu_sb[:], 0.25)
            wu_ps = psw.tile([128, 128], f32, name="wups")
            for _ in range(NWARM):
                nc.tensor.matmul(
                    wu_ps[:], lhsT=wu_sb[:], rhs=wu_sb[:], start=True, stop=True
                )
            nc.vector.tensor_copy(out=wu_sb[:, 0:8], in_=wu_ps[:, 0:8])

            xt_tiles = {}
            w_q = {}

            def issue_load(g):
                xt = xpool.tile([128, GRP, 2, N], f16)
                nc.sync.dma_start(
                    xt[:], x_d[g].rearrange("p (i a w) -> p i a w", i=GRP, a=2)
                )
                xt_tiles[g] = xt

            def issue_wq(q):
                wq = cpool.tile([128, 2, 2, N], f16, name=f"wq{q}")
                nc.sync.dma_start(wq[:], w_d[:, 2 * q : 2 * (q + 1)])
                w_q[q] = wq

            xh_tiles = {}

            def issue_load_half(g, h):
                xt = xpool.tile(
                    [128, 2, 2, N], f16, tag="xh", name=f"xh{g}_{h}",
                    padded_shape=[128, GRP, 2, N],
                )
                nc.sync.dma_start(
                    xt[:],
                    x_d[g].rearrange("p (i a w) -> p i a w", i=GRP, a=2)[
                        :, 2 * h : 2 * h + 2
                    ],
                )
                xh_tiles[(g, h)] = xt

            # one ring, need-order: everything arrives roughly in the
            # order compute consumes it; the first pieces are halved so
            # the very first image + its W complete as early as possible
            issue_wq(0)
            issue_load_half(0, 0)
            issue_load_half(0, 1)
            issue_wq(1)
            issue_load_half(1, 0)
            issue_load_half(1, 1)
            issue_wq(2)
            issue_load(2)
            issue_wq(3)
            issue_load(3)
            issue_wq(4)
            issue_load(4)
            issue_wq(5)
            issue_load(5)
            issue_wq(6)
            issue_load(6)
            issue_wq(7)
            for g in range(7, NG):
                issue_load(g)

            for g in range(NG):
                ot = opool.tile([128, GRP, 2, N], f16)
                for ii in range(GRP):
                    img = g * GRP + ii
                    b = img // CHANNELS
                    wv = w_q[b // 2][:, b % 2]
                    if g < 2:
                        xt = xh_tiles[(g, ii // 2)][:, ii % 2 : ii % 2 + 1]
                    else:
                        xt = xt_tiles[g][:, ii : ii + 1]
                    t1_ps = ps1.tile([128, 2, N], f32)
                    for mb in range(2):
                        for a in range(2):
                            nc.tensor.matmul(
                                t1_ps[:, mb, :],
                                lhsT=xt[:, 0, a, mb * 128 : (mb + 1) * 128],
                                rhs=wv[:, a],
                                start=(a == 0),
                                stop=(a == 1),
                            )
                    t1_sb = tpool.tile([128, 2, N], f16)
                    if ii % 2 == 0:
                        nc.vector.tensor_copy(out=t1_sb[:], in_=t1_ps[:])
                    else:
                        nc.scalar.copy(t1_sb[:], t1_ps[:])
                    t2_ps = ps2.tile([128, 2, N], f32)
                    for mb in range(2):
                        for a in range(2):
                            nc.tensor.matmul(
                                t2_ps[:, mb, :],
                                lhsT=t1_sb[:, a, mb * 128 : (mb + 1) * 128],
                                rhs=wv[:, a],
                                start=(a == 0),
                                stop=(a == 1),
                            )
                    if ii % 2 == 0:
                        nc.scalar.copy(ot[:, ii], t2_ps[:])
                    else:
                        nc.vector.tensor_copy(out=ot[:, ii], in_=t2_ps[:])
                    # late groups: store per image-pair as soon as the
                    # pair is done; the last group per image.  All on the
                    # sync ring, idle once loads finish, so dispatches
                    # never queue behind other work.
                    if g == NG - 1:
                        nc.sync.dma_start(
                            o_d[g].rearrange(
                                "p (i a w) -> p i a w", i=GRP, a=2
                            )[:, ii],
                            ot[:, ii],
                        )
                    elif g >= NG // 2 and ii % 2 == 1:
                        nc.sync.dma_start(
                            o_d[g].rearrange(
                                "p (i a w) -> p i a w", i=GRP, a=2
                            )[:, ii - 1 : ii + 1],
                            ot[:, ii - 1 : ii + 1],
                        )
                if g < NG // 2:
                    nc.scalar.dma_start(
                        o_d[g].rearrange("p (i a w) -> p i a w", i=GRP, a=2), ot[:]
                    )

    nc.compile()
    return nc


def _get_nc():
    key = "nc_v7"
    if key not in _NC_CACHE:
        _NC_CACHE[key] = _build_nc()
    return _NC_CACHE[key]


def _host_w(blur_sigmas, fwd_steps):
    """Per-batch W_b = (D diag(e_b) D)^T in device layout [128, B, 2, N]."""
    sig = np.asarray(blur_sigmas, dtype=np.float64)
    steps = np.asarray(fwd_steps).astype(np.int64)
    n = np.arange(N, dtype=np.float64)
    D = np.sqrt(2.0 / N) * np.cos(np.pi * (n[None, :] + 0.5) * n[:, None] / N)
    D[0] *= 1.0 / np.sqrt(2.0)
    freqs = np.pi * n / N
    uniq, inv = np.unique(steps, return_inverse=True)
    ms = np.empty((len(uniq), N, N), dtype=np.float16)
    for i, s in enumerate(uniq):
        t = sig[s] ** 2 / 2.0
        e = np.exp(-(freqs**2) * t)
        w = (D @ (e[:, None] * D)).T
        ms[i] = w.astype(np.float16)
    w_all = ms[inv]  # [B, N, N]
    # device layout [128, B, 2, N]: [p, b, a, h] = W_b[a*128+p, h]
    return np.ascontiguousarray(
        w_all.reshape(BATCH, 2, 128, N).transpose(2, 0, 1, 3)
    )


def kernel(x, blur_sigmas, fwd_steps):
    global LAST_EXEC_TIME_NS
    from concourse import bass_utils

    x = np.asarray(x)
    assert x.shape == (BATCH, CHANNELS, N, N), x.shape
    x = x.astype(np.float16)
    w_host = _host_w(blur_sigmas, fwd_steps)

    # device x layout: [core][NG, 128, GRP*2*N]
    # x[img, a*128+p, w] -> xc[g, p, (i, a, w)]
    xp = (
        x.reshape(N_CORES, NG, GRP, 2, 128, N)
        .transpose(0, 1, 4, 2, 3, 5)
        .reshape(N_CORES, NG, 128, GRP * 2 * N)
    )
    in_maps = []
    for i in range(N_CORES):
        in_maps.append(
            {
                "x": np.ascontiguousarray(xp[i]),
                "w": np.ascontiguousarray(w_host[:, i * PB : (i + 1) * PB]),
            }
        )

    nc = _get_nc()
    trace = os.environ.get("BASS_DCT_TRACE", "0") == "1"
    kwargs = {}
    if trace:
        _install_ntff_hook()
        kwargs["trace"] = True
        tmpdir = os.environ.get("BASS_DCT_TRACE_DIR")
        if tmpdir:
            kwargs["tmpdir"] = tmpdir
    res = None
    for attempt in range(3):
        try:
            res = bass_utils.run_bass_kernel_spmd(
                nc, in_maps, core_ids=list(range(N_CORES)), **kwargs
            )
            break
        except Exception:
            # transient NRT_EXEC_UNIT_UNRECOVERABLE has been observed on the
            # first execution of a freshly loaded NEFF; a retry succeeds
            if attempt == 2:
                raise
            import time as _time

            _time.sleep(2.0)
            kwargs.pop("trace", None)
            kwargs.pop("tmpdir", None)
    LAST_EXEC_TIME_NS = res.exec_time_ns

    # inverse permute: oc[g, p, (i, a, w)] -> out[img, a*128+p, w]
    oc = np.stack([res.results[i]["o"] for i in range(N_CORES)])
    out = (
        oc.reshape(N_CORES, NG, 128, GRP, 2, N)
        .transpose(0, 1, 3, 4, 2, 5)
        .reshape(BATCH, CHANNELS, N, N)
    )
    return np.ascontiguousarray(out.astype(np.float32))
